# revision 13
# baseline (speedup 1.0000x reference)
"""Causal self-attention (GPT-style block) on 8 Trainium2 NeuronCores.

Problem: x[4,2048,1024] -> qkv = x@W_attn+b ; 16-head causal attention
(head_dim 64) ; out = y@W_proj+b_proj.

Sharding: tensor-parallel over heads. Core c owns heads {2c, 2c+1}:
  - computes q^T/k^T for its heads over the full batch via matmuls against
    a host-pretransposed x^T (bf16); v is computed directly in NATURAL
    layout (rows on partitions) by swapping matmul roles (x^T tile
    stationary, W_v moving) — no DRAM roundtrip / DMA transposes,
  - runs causal attention for its 8 (batch, head) pairs entirely in SBUF
    (S^T layout: scores tile [128 j, 512 i]; both heads' scores share one
    2-bank PSUM tile so a single wide exp on ScalarE covers them; causal
    mask on GpSimd; PV matmul with a ones-column appended to V producing
    both y_raw^T and the softmax denominator in one PSUM tile),
  - qkv compute for batch b+1 is interleaved into the attention blocks of
    batch b so the PE queue never drains (PE p-state ramp: idle resets
    the clock to 0.65GHz, 3us of continuous busy to reach 2.4GHz),
  - S(j) is emitted one tile ahead of PV(j-1) so PV never heads the PE
    queue while its exp is still in flight,
  - the head->row exchange is split per batch: after attn(b) normalizes,
    an AllToAll of [8, 128 feats, 256 rows] ships batch b's rows (core r
    takes rows [b*2048 + r*256, +256) of each batch).  Exchanges 0-2 run
    under later batches' compute; their output-projection tiles (with
    gather DMAs prefetched one slot ahead) interleave into the attention
    blocks of batches 2 and 3.  Only exchange 3 + two projection tiles
    remain as tail.  The host reorders the 32 row blocks at the end.

Numerics: bf16 operands with fp32 PSUM accumulation everywhere; softmax
skips the max-subtraction (scores are O(1) by construction; exp stays
finite) which matches the reference to ~5e-3 in fp32.
"""

import numpy as np
import ml_dtypes
from contextlib import ExitStack

import concourse.bass as bass
import concourse.tile as tile
from concourse import bacc, mybir
from concourse.tile_rust import add_dep_helper
from concourse.bass_utils import run_bass_kernel_spmd

F32 = mybir.dt.float32
BF16 = mybir.dt.bfloat16
AF = mybir.ActivationFunctionType

N_CORES = 8
B, T, C, H = 4, 2048, 1024, 16
HD = C // H            # 64 head dim
HPC = H // N_CORES     # 2 heads per core
FPC = HPC * HD         # 128 features per core
BT = B * T             # 8192 rows
TCHUNK = 512           # t chunk in qkv phase
QB = 512               # query block
NQB = T // QB          # 4 per batch
JTN = T // 128         # 16 j-tiles per batch
ROWS = BT // N_CORES   # 1024 rows per core after the exchanges
RPB = T // N_CORES     # 256 rows per (core, batch)
KC = C // 128          # 8 contraction tiles over C
VW = 80                # [V | 1 | pad] row unit
SCALE = 1.0 / np.sqrt(HD)

LAST_RESULTS = None    # test.py reads exec_time_ns off this


def build_program(nc):
    # weight tensors arrive host-pre-rearranged to the SBUF layout so each
    # load is one partition-contiguous DMA (2-16KB packets, not 256B)
    xT = nc.dram_tensor("xT", [C, BT], BF16, kind="ExternalInput").ap()
    wq = nc.dram_tensor("wq", [128, KC, FPC], BF16, kind="ExternalInput").ap()
    wk = nc.dram_tensor("wk", [128, KC, FPC], BF16, kind="ExternalInput").ap()
    wv = nc.dram_tensor("wv", [128, KC, FPC], BF16, kind="ExternalInput").ap()
    bqkv = nc.dram_tensor("bqkv", [3, FPC], F32, kind="ExternalInput").ap()
    wp = nc.dram_tensor("wp", [128, KC, C], BF16, kind="ExternalInput").ap()
    bp = nc.dram_tensor("bp", [C], F32, kind="ExternalInput").ap()
    out = nc.dram_tensor("out", [ROWS, C], F32, kind="ExternalOutput").ap()
    ccs = []
    for b in range(B):
        ci = nc.dram_tensor(f"cc_in{b}", [N_CORES, FPC, RPB], BF16, kind="Internal").ap()
        co = nc.dram_tensor(f"cc_out{b}", [N_CORES, FPC, RPB], BF16, kind="Internal").ap()
        ccs.append((ci, co))

    with tile.TileContext(nc) as tc:
        with ExitStack() as ctx:
            emit(ctx, tc, xT, wq, wk, wv, bqkv, wp, bp, out, ccs)
    return nc


def emit(ctx, tc, xT, wq, wk, wv, bqkv, wp, bp, out, ccs):
    nc = tc.nc
    res = ctx.enter_context(tc.tile_pool(name="resident", bufs=1))

    # ---------- resident SBUF ----------
    qT = res.tile([128, BT], BF16)
    kT = res.tile([128, BT], BF16)
    vsb = res.tile([128, B * JTN, HPC, VW], BF16)     # [V | 1 | pad] per j-tile/head
    wq_sb = res.tile([128, KC, FPC], BF16)
    wk_sb = res.tile([128, KC, FPC], BF16)
    wv_sb = res.tile([128, KC, FPC], BF16)
    b_sb = res.tile([128, 3], F32)
    bv_sb = res.tile([128, 4, HPC, HD], F32)          # v-bias, free-axis broadcast
    wp_sb = res.tile([128, KC, C], BF16)
    bp_sb = res.tile([128, C], F32)
    yT0 = res.tile([64, BT], BF16)
    yT1 = res.tile([64, BT], BF16)
    yT = (yT0, yT1)

    # ---------- constant/weight loads (wp/bp deferred past warmup) ----------
    nc.sync.dma_start(wq_sb[:], wq)
    nc.sync.dma_start(b_sb[:], bqkv.rearrange("b p -> p b"))
    nc.sync.dma_start(wk_sb[:], wk)
    nc.sync.dma_start(wv_sb[:], wv)
    bv_bcast = bass.AP(
        tensor=bqkv.tensor, offset=bqkv.offset + 2 * FPC,
        ap=[[0, 128], [0, 4], [HD, HPC], [1, HD]],
    )
    nc.sync.dma_start(bv_sb[:], bv_bcast)
    nc.vector.memset(vsb[:, :, :, HD : HD + 1], 1.0)

    ospool = ctx.enter_context(tc.tile_pool(name="osb", bufs=3))
    yfpool = ctx.enter_context(tc.tile_pool(name="yf", bufs=3))
    ph = ExitStack()
    xpool = ph.enter_context(tc.tile_pool(name="xt", bufs=3))
    gpool = ph.enter_context(tc.tile_pool(name="gps", bufs=2, space="PSUM"))
    spool = ph.enter_context(tc.tile_pool(name="sps", bufs=2, space="PSUM"))
    ypool = ph.enter_context(tc.tile_pool(name="yps", bufs=2, space="PSUM"))
    ptpool = ph.enter_context(tc.tile_pool(name="pt", bufs=3))
    npool = ph.enter_context(tc.tile_pool(name="norm", bufs=3))

    xT_t = xT.rearrange("(a p) t -> p a t", p=128)
    staging = [[] for _ in range(B)]

    def emit_qkv_chunk(tci):
        """q^T/k^T (weights stationary) + natural-layout v (x^T stationary)
        for one 512-row chunk."""
        t0 = tci * TCHUNK
        xt = xpool.tile([128, KC, TCHUNK], BF16, tag="xt", name=f"xt{tci}")
        for spl in range(4):
            nc.sync.dma_start(
                xt[:, 2 * spl : 2 * spl + 2, :],
                xT_t[:, 2 * spl : 2 * spl + 2, t0 : t0 + TCHUNK],
            )
        for w_sb, bi, dst in ((wq_sb, 0, qT), (wk_sb, 1, kT)):
            ps = gpool.tile([128, TCHUNK], F32, tag="g", name=f"ps{tci}_{bi}")
            for a in range(KC):
                nc.tensor.matmul(
                    ps[:], lhsT=w_sb[:, a, :], rhs=xt[:, a, :],
                    start=(a == 0), stop=(a == KC - 1),
                )
            nc.vector.tensor_scalar_add(
                dst[:, t0 : t0 + TCHUNK], ps[:], b_sb[:, bi : bi + 1]
            )
        vp = gpool.tile([128, 4, HPC, HD], F32, tag="g", name=f"vp{tci}")
        for sub in range(4):
            for a in range(KC):
                nc.tensor.matmul(
                    vp[:, sub], lhsT=xt[:, a, sub * 128 : (sub + 1) * 128],
                    rhs=wv_sb[:, a, :], start=(a == 0), stop=(a == KC - 1),
                )
        g0 = tci * 4
        nc.vector.tensor_add(vsb[:, g0 : g0 + 4, :, 0:HD], vp[:], bv_sb[:])

    def emit_pv(b, j, pt, yps, njt):
        for h in range(HPC):
            nc.tensor.matmul(
                yps[h][:], lhsT=vsb[:, b * JTN + j, h, 0 : HD + 1],
                rhs=pt[:, h, :], start=(j == 0), stop=(j == njt - 1),
            )

    def emit_attn_block(b, qb):
        """Causal attention for one (batch, query-block): S^T for both heads
        into a 2-bank PSUM pair, one wide exp, PV one j behind S."""
        q0g = b * T + qb * QB
        njt = (qb + 1) * (QB // 128)
        yps = [
            ypool.tile([HD + 1, QB], F32, tag="yp", name=f"yp{b}_{qb}_{h}")
            for h in range(HPC)
        ]
        pts = []
        for j in range(njt):
            j0g = b * T + j * 128
            diag = j * 128 + 127 > qb * QB
            i0 = max(0, j * 128 - qb * QB)
            sp = spool.tile([128, HPC, QB], F32, tag="sp", name=f"sp{b}_{qb}_{j}")
            for h in range(HPC):
                hs = slice(h * HD, (h + 1) * HD)
                nc.tensor.matmul(
                    sp[:, h, i0:QB], lhsT=kT[hs, j0g : j0g + 128],
                    rhs=qT[hs, q0g + i0 : q0g + QB], start=True, stop=True,
                )
            pt = ptpool.tile([128, HPC, QB], BF16, tag="pt", name=f"pt{b}_{qb}_{j}")
            if i0:
                nc.vector.memset(pt[:, :, 0:i0], 0.0)
            nc.scalar.activation(
                pt[:, :, i0:QB], sp[:, :, i0:QB], AF.Exp, scale=float(SCALE)
            )
            if diag:
                for h in range(HPC):
                    nc.gpsimd.affine_select(
                        pt[:, h, i0 : i0 + 128], pt[:, h, i0 : i0 + 128],
                        pattern=[[1, 128]], base=0, channel_multiplier=-1,
                        compare_op=mybir.AluOpType.is_ge, fill=0.0,
                    )
            pts.append(pt)
            if j > 0:
                emit_pv(b, j - 1, pts[j - 1], yps, njt)
        emit_pv(b, njt - 1, pts[njt - 1], yps, njt)

        # softmax normalization (row HD of yp is the denominator), then stage
        # the two 256-row exchange slices of this block (plain DMA).
        cc_in = ccs[b][0]
        for h in range(HPC):
            ln = npool.tile([1, QB], F32, tag="ln")
            nc.vector.tensor_copy(ln[:], yps[h][HD : HD + 1, :])
            yraw = npool.tile([HD, QB], F32, tag="yraw")
            nc.vector.tensor_copy(yraw[:], yps[h][0:HD, :])
            rn = npool.tile([1, QB], F32, tag="rn")
            sc = npool.tile([1, QB], F32, tag="sc")
            nc.vector.reciprocal_approx_accurate(rn[:], ln[:], sc[:])
            rb = npool.tile([HD, QB], F32, tag="rb")
            nc.gpsimd.partition_broadcast(rb[:], rn[:], channels=HD)
            nc.vector.tensor_mul(yT[h][:, q0g : q0g + QB], yraw[:], rb[:])
            for half in range(2):
                ti = nc.sync.dma_start(
                    cc_in[2 * qb + half, h * HD : (h + 1) * HD, :],
                    yT[h][:, q0g + half * RPB : q0g + (half + 1) * RPB],
                )
                staging[b].append(ti)

    def emit_exchange(b):
        cc_in, cc_out = ccs[b]
        cci = nc.gpsimd.collective_compute(
            "AllToAll", mybir.AluOpType.bypass,
            ins=[cc_in[:]], outs=[cc_out[:]],
            replica_groups=[list(range(N_CORES))],
        )
        for ti in staging[b]:
            add_dep_helper(cci.ins, ti.ins, True, f"staging before exch{b}")

    pending_proj = []  # (yf_tile, out_row0) with gather DMA already issued

    def emit_proj_dma(b, half):
        """Prefetch the gathered 128-row y tile for proj tile (b, half)."""
        cc_out = ccs[b][1]
        yf = yfpool.tile([128, KC, 128], BF16, tag="yf", name=f"yf{b}_{half}")
        src = bass.AP(
            tensor=cc_out.tensor, offset=cc_out.offset + half * 128,
            ap=[[RPB, 128], [FPC * RPB, N_CORES], [1, 128]],
        )
        nc.sync.dma_start(yf[:], src)
        pending_proj.append((yf, b * RPB + half * 128))

    def emit_proj_mm(psum_pool, ptag):
        """Output projection for the oldest prefetched tile."""
        yf, row0 = pending_proj.pop(0)
        ps0 = psum_pool.tile([128, 512], F32, tag=ptag, name=f"op0_{row0}")
        ps1 = psum_pool.tile([128, 512], F32, tag=ptag, name=f"op1_{row0}")
        for a in range(KC):
            nc.tensor.matmul(ps0[:], lhsT=yf[:, a, :], rhs=wp_sb[:, a, 0:512],
                             start=(a == 0), stop=(a == KC - 1))
            nc.tensor.matmul(ps1[:], lhsT=yf[:, a, :], rhs=wp_sb[:, a, 512:C],
                             start=(a == 0), stop=(a == KC - 1))
        osb = ospool.tile([128, C], F32, tag="osb", name=f"osb{row0}")
        nc.vector.tensor_add(osb[:, 0:512], ps0[:], bp_sb[:, 0:512])
        nc.vector.tensor_add(osb[:, 512:C], ps1[:], bp_sb[:, 512:C])
        nc.sync.dma_start(out[row0 : row0 + 128, :], osb[:])

    # ---------- fused qkv + attention + exchange + projection pipeline ----------
    # batch 0's chunks interleave with its own attention (one chunk ahead);
    # later batches' chunks ride inside the previous batch's blocks.
    emit_qkv_chunk(0)
    nc.sync.dma_start(wp_sb[:], wp)
    bp_bcast = bass.AP(tensor=bp.tensor, offset=bp.offset, ap=[[0, 128], [1, C]])
    nc.sync.dma_start(bp_sb[:], bp_bcast)

    # proj work scheduled into the attention blocks of batches 2 and 3:
    # (dma, mm) pairs per (b, qb) slot; dmas run one slot ahead of their mms.
    proj_slots = {
        (2, 0): ([(0, 0)], 0), (2, 1): ([(0, 1)], 1), (2, 2): ([(1, 0)], 1),
        (2, 3): ([(1, 1)], 1), (3, 0): ([(2, 0)], 1), (3, 1): ([(2, 1)], 1),
        (3, 2): ([], 1), (3, 3): ([], 0),
    }
    for b in range(B):
        for qb in range(NQB):
            if b == 0 and qb + 1 < NQB:
                emit_qkv_chunk(qb + 1)
            emit_attn_block(b, qb)
            if b + 1 < B:
                emit_qkv_chunk((b + 1) * NQB + qb)
            dmas, n_mm = proj_slots.get((b, qb), ([], 0))
            for pb, half in dmas:
                emit_proj_dma(pb, half)
            for _ in range(n_mm):
                emit_proj_mm(gpool, "g")
        emit_exchange(b)

    # tail: batch-3 exchange + its two projection tiles
    emit_proj_dma(3, 0)
    emit_proj_dma(3, 1)
    emit_proj_mm(gpool, "g")
    emit_proj_mm(gpool, "g")
    assert not pending_proj
    ph.close()


_COMPILED_NC = None


def _get_nc():
    global _COMPILED_NC
    if _COMPILED_NC is None:
        nc = bacc.Bacc("TRN2", target_bir_lowering=False, debug=False,
                       num_devices=N_CORES)
        build_program(nc)
        nc.compile()
    _COMPILED_NC = nc
    return _COMPILED_NC


def kernel(x, W_attn, b_attn, W_proj, b_proj):
    global LAST_RESULTS
    nc = _get_nc()

    bf = ml_dtypes.bfloat16
    xT_np = np.ascontiguousarray(
        np.asarray(x, np.float32).reshape(BT, C).T
    ).astype(bf)
    W_attn = np.asarray(W_attn, np.float32)
    b_attn = np.asarray(b_attn, np.float32)
    wp_np = np.ascontiguousarray(
        np.asarray(W_proj, np.float32).reshape(KC, 128, C).transpose(1, 0, 2)
    ).astype(bf)
    bp_np = np.asarray(b_proj, np.float32)

    def sb_layout(w, ncols):
        # [C, ncols] -> [128, KC, ncols]: partition-contiguous SBUF layout
        return np.ascontiguousarray(
            w.reshape(KC, 128, ncols).transpose(1, 0, 2)
        ).astype(bf)

    in_maps = []
    for c in range(N_CORES):
        s = slice(c * FPC, (c + 1) * FPC)
        in_maps.append({
            "xT": xT_np,
            "wq": sb_layout(W_attn[:, s], FPC),
            "wk": sb_layout(W_attn[:, C:2 * C][:, s], FPC),
            "wv": sb_layout(W_attn[:, 2 * C:][:, s], FPC),
            "bqkv": np.ascontiguousarray(
                np.stack([b_attn[s], b_attn[C:2 * C][s], b_attn[2 * C:][s]])
            ).astype(np.float32),
            "wp": wp_np,
            "bp": bp_np,
        })

    res = run_bass_kernel_spmd(nc, in_maps, core_ids=list(range(N_CORES)))
    LAST_RESULTS = res
    # Core r holds, per batch b, global rows [b*T + r*RPB, +RPB) at local
    # rows [b*RPB, +RPB).
    full = np.empty((BT, C), np.float32)
    for b in range(B):
        for r in range(N_CORES):
            full[b * T + r * RPB : b * T + (r + 1) * RPB] = (
                res.results[r]["out"][b * RPB : (b + 1) * RPB]
            )
    return full.reshape(B, T, C)


# revision 14
# speedup vs baseline: 1.0184x; 1.0184x over previous
"""Causal self-attention (GPT-style block) on 8 Trainium2 NeuronCores.

Problem: x[4,2048,1024] -> qkv = x@W_attn+b ; 16-head causal attention
(head_dim 64) ; out = y@W_proj+b_proj.

Sharding: tensor-parallel over heads. Core c owns heads {2c, 2c+1}:
  - computes q^T/k^T for its heads over the full batch via matmuls against
    a host-pretransposed x^T (bf16); v is computed directly in NATURAL
    layout (rows on partitions) by swapping matmul roles (x^T tile
    stationary, W_v moving) — no DRAM roundtrip / DMA transposes,
  - runs causal attention for its 8 (batch, head) pairs entirely in SBUF
    (S^T layout: scores tile [128 j, 512 i]; both heads' scores share one
    2-bank PSUM tile so a single wide exp on ScalarE covers them; causal
    mask on GpSimd; PV matmul with a ones-column appended to V producing
    both y_raw^T and the softmax denominator in one PSUM tile),
  - qkv compute for batch b+1 is interleaved into the attention blocks of
    batch b so the PE queue never drains (PE p-state ramp: idle resets
    the clock to 0.65GHz, 3us of continuous busy to reach 2.4GHz),
  - S(j) is emitted one tile ahead of PV(j-1) so PV never heads the PE
    queue while its exp is still in flight,
  - the head->row exchange is split per batch: after attn(b) normalizes,
    an AllToAll of [8, 128 feats, 256 rows] ships batch b's rows (core r
    takes rows [b*2048 + r*256, +256) of each batch).  Exchanges 0-2 run
    under later batches' compute; their output-projection tiles (with
    gather DMAs prefetched one slot ahead) interleave into the attention
    blocks of batches 2 and 3.  Only exchange 3 + two projection tiles
    remain as tail.  The host reorders the 32 row blocks at the end.

Numerics: bf16 operands with fp32 PSUM accumulation everywhere; softmax
skips the max-subtraction (scores are O(1) by construction; exp stays
finite) which matches the reference to ~5e-3 in fp32.
"""

import numpy as np
import ml_dtypes
from contextlib import ExitStack

import concourse.bass as bass
import concourse.tile as tile
from concourse import bacc, mybir
from concourse.tile_rust import add_dep_helper
from concourse.bass_utils import run_bass_kernel_spmd

F32 = mybir.dt.float32
BF16 = mybir.dt.bfloat16
AF = mybir.ActivationFunctionType

N_CORES = 8
B, T, C, H = 4, 2048, 1024, 16
HD = C // H            # 64 head dim
HPC = H // N_CORES     # 2 heads per core
FPC = HPC * HD         # 128 features per core
BT = B * T             # 8192 rows
TCHUNK = 512           # t chunk in qkv phase
QB = 512               # query block
NQB = T // QB          # 4 per batch
JTN = T // 128         # 16 j-tiles per batch
ROWS = BT // N_CORES   # 1024 rows per core after the exchanges
RPB = T // N_CORES     # 256 rows per (core, batch)
KC = C // 128          # 8 contraction tiles over C
VW = 80                # [V | 1 | pad] row unit
SCALE = 1.0 / np.sqrt(HD)

LAST_RESULTS = None    # test.py reads exec_time_ns off this


def build_program(nc):
    # weight tensors arrive host-pre-rearranged to the SBUF layout so each
    # load is one partition-contiguous DMA (2-16KB packets, not 256B)
    xT = nc.dram_tensor("xT", [C, BT], BF16, kind="ExternalInput").ap()
    wq = nc.dram_tensor("wq", [128, KC, FPC], BF16, kind="ExternalInput").ap()
    wk = nc.dram_tensor("wk", [128, KC, FPC], BF16, kind="ExternalInput").ap()
    wv = nc.dram_tensor("wv", [128, KC, FPC], BF16, kind="ExternalInput").ap()
    bqkv = nc.dram_tensor("bqkv", [3, FPC], F32, kind="ExternalInput").ap()
    wp = nc.dram_tensor("wp", [128, KC, C], BF16, kind="ExternalInput").ap()
    bp = nc.dram_tensor("bp", [C], F32, kind="ExternalInput").ap()
    out = nc.dram_tensor("out", [ROWS, C], F32, kind="ExternalOutput").ap()
    ccs = []
    for b in range(B):
        ci = nc.dram_tensor(f"cc_in{b}", [N_CORES, FPC, RPB], BF16, kind="Internal").ap()
        co = nc.dram_tensor(f"cc_out{b}", [N_CORES, FPC, RPB], BF16, kind="Internal").ap()
        ccs.append((ci, co))

    with tile.TileContext(nc) as tc:
        with ExitStack() as ctx:
            emit(ctx, tc, xT, wq, wk, wv, bqkv, wp, bp, out, ccs)
    return nc


def emit(ctx, tc, xT, wq, wk, wv, bqkv, wp, bp, out, ccs):
    nc = tc.nc
    res = ctx.enter_context(tc.tile_pool(name="resident", bufs=1))

    # ---------- resident SBUF ----------
    qT = res.tile([128, BT], BF16)
    kT = res.tile([128, BT], BF16)
    vsb = res.tile([128, B * JTN, HPC, VW], BF16)     # [V | 1 | pad] per j-tile/head
    wq_sb = res.tile([128, KC, FPC], BF16)
    wk_sb = res.tile([128, KC, FPC], BF16)
    wv_sb = res.tile([128, KC, FPC], BF16)
    b_sb = res.tile([128, 3], F32)
    bv_sb = res.tile([128, 4, HPC, HD], F32)          # v-bias, free-axis broadcast
    wp_sb = res.tile([128, KC, C], BF16)
    bp_sb = res.tile([128, C], F32)
    yT0 = res.tile([64, BT], BF16)
    yT1 = res.tile([64, BT], BF16)
    yT = (yT0, yT1)

    # ---------- constant/weight loads (wp/bp deferred past warmup) ----------
    nc.sync.dma_start(wq_sb[:], wq)
    nc.sync.dma_start(b_sb[:], bqkv.rearrange("b p -> p b"))
    nc.sync.dma_start(wk_sb[:], wk)
    nc.sync.dma_start(wv_sb[:], wv)
    bv_bcast = bass.AP(
        tensor=bqkv.tensor, offset=bqkv.offset + 2 * FPC,
        ap=[[0, 128], [0, 4], [HD, HPC], [1, HD]],
    )
    nc.sync.dma_start(bv_sb[:], bv_bcast)
    nc.vector.memset(vsb[:, :, :, HD : HD + 1], 1.0)

    ospool = ctx.enter_context(tc.tile_pool(name="osb", bufs=3))
    yfpool = ctx.enter_context(tc.tile_pool(name="yf", bufs=3))
    ph = ExitStack()
    xpool = ph.enter_context(tc.tile_pool(name="xt", bufs=3))
    gpool = ph.enter_context(tc.tile_pool(name="gps", bufs=2, space="PSUM"))
    spool = ph.enter_context(tc.tile_pool(name="sps", bufs=2, space="PSUM"))
    ypool = ph.enter_context(tc.tile_pool(name="yps", bufs=2, space="PSUM"))
    ptpool = ph.enter_context(tc.tile_pool(name="pt", bufs=3))
    npool = ph.enter_context(tc.tile_pool(name="norm", bufs=3))

    xT_t = xT.rearrange("(a p) t -> p a t", p=128)
    staging = [[] for _ in range(B)]

    def emit_qkv_chunk(tci):
        """q^T/k^T (weights stationary) + natural-layout v (x^T stationary)
        for one 512-row chunk."""
        t0 = tci * TCHUNK
        xt = xpool.tile([128, KC, TCHUNK], BF16, tag="xt", name=f"xt{tci}")
        for spl in range(4):
            nc.sync.dma_start(
                xt[:, 2 * spl : 2 * spl + 2, :],
                xT_t[:, 2 * spl : 2 * spl + 2, t0 : t0 + TCHUNK],
            )
        for w_sb, bi, dst in ((wq_sb, 0, qT), (wk_sb, 1, kT)):
            ps = gpool.tile([128, TCHUNK], F32, tag="g", name=f"ps{tci}_{bi}")
            for a in range(KC):
                nc.tensor.matmul(
                    ps[:], lhsT=w_sb[:, a, :], rhs=xt[:, a, :],
                    start=(a == 0), stop=(a == KC - 1),
                )
            nc.vector.tensor_scalar_add(
                dst[:, t0 : t0 + TCHUNK], ps[:], b_sb[:, bi : bi + 1]
            )
        vp = gpool.tile([128, 4, HPC, HD], F32, tag="g", name=f"vp{tci}")
        for sub in range(4):
            for a in range(KC):
                nc.tensor.matmul(
                    vp[:, sub], lhsT=xt[:, a, sub * 128 : (sub + 1) * 128],
                    rhs=wv_sb[:, a, :], start=(a == 0), stop=(a == KC - 1),
                )
        g0 = tci * 4
        nc.vector.tensor_add(vsb[:, g0 : g0 + 4, :, 0:HD], vp[:], bv_sb[:])

    def emit_pv(b, j, pt, yps, njt):
        for h in range(HPC):
            nc.tensor.matmul(
                yps[h][:], lhsT=vsb[:, b * JTN + j, h, 0 : HD + 1],
                rhs=pt[:, h, :], start=(j == 0), stop=(j == njt - 1),
            )

    def emit_attn_block(b, qb):
        """Causal attention for one (batch, query-block): S^T for both heads
        into a 2-bank PSUM pair, one wide exp, PV one j behind S."""
        q0g = b * T + qb * QB
        njt = (qb + 1) * (QB // 128)
        yps = [
            ypool.tile([HD + 1, QB], F32, tag="yp", name=f"yp{b}_{qb}_{h}")
            for h in range(HPC)
        ]
        pts = []
        for j in range(njt):
            j0g = b * T + j * 128
            diag = j * 128 + 127 > qb * QB
            i0 = max(0, j * 128 - qb * QB)
            sp = spool.tile([128, HPC, QB], F32, tag="sp", name=f"sp{b}_{qb}_{j}")
            for h in range(HPC):
                hs = slice(h * HD, (h + 1) * HD)
                nc.tensor.matmul(
                    sp[:, h, i0:QB], lhsT=kT[hs, j0g : j0g + 128],
                    rhs=qT[hs, q0g + i0 : q0g + QB], start=True, stop=True,
                )
            pt = ptpool.tile([128, HPC, QB], BF16, tag="pt", name=f"pt{b}_{qb}_{j}")
            if i0:
                nc.vector.memset(pt[:, :, 0:i0], 0.0)
            nc.scalar.activation(
                pt[:, :, i0:QB], sp[:, :, i0:QB], AF.Exp, scale=float(SCALE)
            )
            if diag:
                for h in range(HPC):
                    nc.gpsimd.affine_select(
                        pt[:, h, i0 : i0 + 128], pt[:, h, i0 : i0 + 128],
                        pattern=[[1, 128]], base=0, channel_multiplier=-1,
                        compare_op=mybir.AluOpType.is_ge, fill=0.0,
                    )
            pts.append(pt)
            if j > 0:
                emit_pv(b, j - 1, pts[j - 1], yps, njt)
        emit_pv(b, njt - 1, pts[njt - 1], yps, njt)

        # softmax normalization (row HD of yp is the denominator), then stage
        # the two 256-row exchange slices of this block (plain DMA).
        cc_in = ccs[b][0]
        for h in range(HPC):
            ln = npool.tile([1, QB], F32, tag="ln")
            nc.vector.tensor_copy(ln[:], yps[h][HD : HD + 1, :])
            yraw = npool.tile([HD, QB], F32, tag="yraw")
            nc.vector.tensor_copy(yraw[:], yps[h][0:HD, :])
            rn = npool.tile([1, QB], F32, tag="rn")
            sc = npool.tile([1, QB], F32, tag="sc")
            nc.vector.reciprocal_approx_accurate(rn[:], ln[:], sc[:])
            rb = npool.tile([HD, QB], F32, tag="rb")
            nc.gpsimd.partition_broadcast(rb[:], rn[:], channels=HD)
            nc.vector.tensor_mul(yT[h][:, q0g : q0g + QB], yraw[:], rb[:])
            for half in range(2):
                ti = nc.sync.dma_start(
                    cc_in[2 * qb + half, h * HD : (h + 1) * HD, :],
                    yT[h][:, q0g + half * RPB : q0g + (half + 1) * RPB],
                )
                staging[b].append(ti)

    def emit_exchange(b):
        cc_in, cc_out = ccs[b]
        cci = nc.gpsimd.collective_compute(
            "AllToAll", mybir.AluOpType.bypass,
            ins=[cc_in[:]], outs=[cc_out[:]],
            replica_groups=[list(range(N_CORES))],
        )
        for ti in staging[b]:
            add_dep_helper(cci.ins, ti.ins, True, f"staging before exch{b}")

    pending_proj = []  # (yf_tile, out_row0) with gather DMA already issued

    def emit_proj_dma(b, half):
        """Prefetch the gathered 128-row y tile for proj tile (b, half)."""
        cc_out = ccs[b][1]
        yf = yfpool.tile([128, KC, 128], BF16, tag="yf", name=f"yf{b}_{half}")
        src = bass.AP(
            tensor=cc_out.tensor, offset=cc_out.offset + half * 128,
            ap=[[RPB, 128], [FPC * RPB, N_CORES], [1, 128]],
        )
        nc.sync.dma_start(yf[:], src)
        pending_proj.append((yf, b * RPB + half * 128))

    def emit_proj_mm(psum_pool, ptag):
        """Output projection for the oldest prefetched tile."""
        yf, row0 = pending_proj.pop(0)
        ps0 = psum_pool.tile([128, 512], F32, tag=ptag, name=f"op0_{row0}")
        ps1 = psum_pool.tile([128, 512], F32, tag=ptag, name=f"op1_{row0}")
        for a in range(KC):
            nc.tensor.matmul(ps0[:], lhsT=yf[:, a, :], rhs=wp_sb[:, a, 0:512],
                             start=(a == 0), stop=(a == KC - 1))
            nc.tensor.matmul(ps1[:], lhsT=yf[:, a, :], rhs=wp_sb[:, a, 512:C],
                             start=(a == 0), stop=(a == KC - 1))
        osb = ospool.tile([128, C], F32, tag="osb", name=f"osb{row0}")
        nc.vector.tensor_add(osb[:, 0:512], ps0[:], bp_sb[:, 0:512])
        nc.vector.tensor_add(osb[:, 512:C], ps1[:], bp_sb[:, 512:C])
        nc.sync.dma_start(out[row0 : row0 + 128, :], osb[:])

    # ---------- fused qkv + attention + exchange + projection pipeline ----------
    # batch 0's chunks interleave with its own attention (one chunk ahead);
    # later batches' chunks ride inside the previous batch's blocks.
    emit_qkv_chunk(0)
    nc.sync.dma_start(wp_sb[:], wp)
    bp_bcast = bass.AP(tensor=bp.tensor, offset=bp.offset, ap=[[0, 128], [1, C]])
    nc.sync.dma_start(bp_sb[:], bp_bcast)

    # proj work scheduled into the attention blocks of batches 2 and 3:
    # (dma, mm) pairs per (b, qb) slot; dmas run one slot ahead of their mms.
    proj_slots = {
        (2, 0): ([(0, 0)], 0), (2, 1): ([(0, 1)], 1), (2, 2): ([(1, 0)], 1),
        (2, 3): ([(1, 1)], 1), (3, 0): ([], 1), (3, 1): ([(2, 0)], 0),
        (3, 2): ([(2, 1)], 1), (3, 3): ([], 1),
    }
    for b in range(B):
        for qb in range(NQB):
            if b == 0 and qb + 1 < NQB:
                emit_qkv_chunk(qb + 1)
            emit_attn_block(b, qb)
            if b + 1 < B:
                emit_qkv_chunk((b + 1) * NQB + qb)
            dmas, n_mm = proj_slots.get((b, qb), ([], 0))
            for pb, half in dmas:
                emit_proj_dma(pb, half)
            for _ in range(n_mm):
                emit_proj_mm(gpool, "g")
        emit_exchange(b)

    # tail: batch-3 exchange + its two projection tiles
    emit_proj_dma(3, 0)
    emit_proj_dma(3, 1)
    emit_proj_mm(gpool, "g")
    emit_proj_mm(gpool, "g")
    assert not pending_proj
    ph.close()


_COMPILED_NC = None


def _get_nc():
    global _COMPILED_NC
    if _COMPILED_NC is None:
        nc = bacc.Bacc("TRN2", target_bir_lowering=False, debug=False,
                       num_devices=N_CORES)
        build_program(nc)
        nc.compile()
    _COMPILED_NC = nc
    return _COMPILED_NC


def kernel(x, W_attn, b_attn, W_proj, b_proj):
    global LAST_RESULTS
    nc = _get_nc()

    bf = ml_dtypes.bfloat16
    xT_np = np.ascontiguousarray(
        np.asarray(x, np.float32).reshape(BT, C).T
    ).astype(bf)
    W_attn = np.asarray(W_attn, np.float32)
    b_attn = np.asarray(b_attn, np.float32)
    wp_np = np.ascontiguousarray(
        np.asarray(W_proj, np.float32).reshape(KC, 128, C).transpose(1, 0, 2)
    ).astype(bf)
    bp_np = np.asarray(b_proj, np.float32)

    def sb_layout(w, ncols):
        # [C, ncols] -> [128, KC, ncols]: partition-contiguous SBUF layout
        return np.ascontiguousarray(
            w.reshape(KC, 128, ncols).transpose(1, 0, 2)
        ).astype(bf)

    in_maps = []
    for c in range(N_CORES):
        s = slice(c * FPC, (c + 1) * FPC)
        in_maps.append({
            "xT": xT_np,
            "wq": sb_layout(W_attn[:, s], FPC),
            "wk": sb_layout(W_attn[:, C:2 * C][:, s], FPC),
            "wv": sb_layout(W_attn[:, 2 * C:][:, s], FPC),
            "bqkv": np.ascontiguousarray(
                np.stack([b_attn[s], b_attn[C:2 * C][s], b_attn[2 * C:][s]])
            ).astype(np.float32),
            "wp": wp_np,
            "bp": bp_np,
        })

    res = run_bass_kernel_spmd(nc, in_maps, core_ids=list(range(N_CORES)))
    LAST_RESULTS = res
    # Core r holds, per batch b, global rows [b*T + r*RPB, +RPB) at local
    # rows [b*RPB, +RPB).
    full = np.empty((BT, C), np.float32)
    for b in range(B):
        for r in range(N_CORES):
            full[b * T + r * RPB : b * T + (r + 1) * RPB] = (
                res.results[r]["out"][b * RPB : (b + 1) * RPB]
            )
    return full.reshape(B, T, C)


# revision 20
# speedup vs baseline: 1.0222x; 1.0037x over previous
"""Causal self-attention (GPT-style block) on 8 Trainium2 NeuronCores.

Problem: x[4,2048,1024] -> qkv = x@W_attn+b ; 16-head causal attention
(head_dim 64) ; out = y@W_proj+b_proj.

Sharding: tensor-parallel over heads. Core c owns heads {2c, 2c+1}:
  - computes q^T/k^T for its heads over the full batch via matmuls against
    a host-pretransposed x^T (bf16); v is computed directly in NATURAL
    layout (rows on partitions) by swapping matmul roles (x^T tile
    stationary, W_v moving) — no DRAM roundtrip / DMA transposes,
  - runs causal attention for its 8 (batch, head) pairs entirely in SBUF
    (S^T layout: scores tile [128 j, 512 i]; both heads' scores share one
    2-bank PSUM tile so a single wide exp on ScalarE covers them; causal
    mask on GpSimd; PV matmul with a ones-column appended to V producing
    both y_raw^T and the softmax denominator in one PSUM tile),
  - qkv compute for batch b+1 is interleaved into the attention blocks of
    batch b so the PE queue never drains (PE p-state ramp: idle resets
    the clock to 0.65GHz, 3us of continuous busy to reach 2.4GHz),
  - S(j) is emitted one tile ahead of PV(j-1) so PV never heads the PE
    queue while its exp is still in flight,
  - the head->row exchange is split per batch: after attn(b) normalizes,
    an AllToAll of [8, 128 feats, 256 rows] ships batch b's rows (core r
    takes rows [b*2048 + r*256, +256) of each batch).  Exchanges 0-2 run
    under later batches' compute; their output-projection tiles (with
    gather DMAs prefetched one slot ahead) interleave into the attention
    blocks of batches 2 and 3.  Only exchange 3 + two projection tiles
    remain as tail.  The host reorders the 32 row blocks at the end.

Numerics: bf16 operands with fp32 PSUM accumulation everywhere; softmax
skips the max-subtraction (scores are O(1) by construction; exp stays
finite) which matches the reference to ~5e-3 in fp32.
"""

import numpy as np
import ml_dtypes
from contextlib import ExitStack

import concourse.bass as bass
import concourse.tile as tile
from concourse import bacc, mybir
from concourse.tile_rust import add_dep_helper
from concourse.bass_utils import run_bass_kernel_spmd

F32 = mybir.dt.float32
BF16 = mybir.dt.bfloat16
AF = mybir.ActivationFunctionType

N_CORES = 8
B, T, C, H = 4, 2048, 1024, 16
HD = C // H            # 64 head dim
HPC = H // N_CORES     # 2 heads per core
FPC = HPC * HD         # 128 features per core
BT = B * T             # 8192 rows
TCHUNK = 512           # t chunk in qkv phase
QB = 512               # query block
NQB = T // QB          # 4 per batch
JTN = T // 128         # 16 j-tiles per batch
ROWS = BT // N_CORES   # 1024 rows per core after the exchanges
RPB = T // N_CORES     # 256 rows per (core, batch)
KC = C // 128          # 8 contraction tiles over C
VW = 80                # [V | 1 | pad] row unit
SCALE = 1.0 / np.sqrt(HD)

LAST_RESULTS = None    # test.py reads exec_time_ns off this


def build_program(nc):
    # weight tensors arrive host-pre-rearranged to the SBUF layout so each
    # load is one partition-contiguous DMA (2-16KB packets, not 256B)
    xT = nc.dram_tensor("xT", [C, BT], BF16, kind="ExternalInput").ap()
    wq = nc.dram_tensor("wq", [128, KC, FPC], BF16, kind="ExternalInput").ap()
    wk = nc.dram_tensor("wk", [128, KC, FPC], BF16, kind="ExternalInput").ap()
    wv = nc.dram_tensor("wv", [128, KC, FPC], BF16, kind="ExternalInput").ap()
    bqkv = nc.dram_tensor("bqkv", [3, FPC], F32, kind="ExternalInput").ap()
    wp = nc.dram_tensor("wp", [128, KC, C], BF16, kind="ExternalInput").ap()
    bp = nc.dram_tensor("bp", [C], F32, kind="ExternalInput").ap()
    out = nc.dram_tensor("out", [ROWS, C], F32, kind="ExternalOutput").ap()
    ccs = []
    for b in range(B):
        ci = nc.dram_tensor(f"cc_in{b}", [N_CORES, FPC, RPB], BF16, kind="Internal").ap()
        co = nc.dram_tensor(f"cc_out{b}", [N_CORES, FPC, RPB], BF16, kind="Internal").ap()
        ccs.append((ci, co))

    with tile.TileContext(nc) as tc:
        with ExitStack() as ctx:
            emit(ctx, tc, xT, wq, wk, wv, bqkv, wp, bp, out, ccs)
    return nc


def emit(ctx, tc, xT, wq, wk, wv, bqkv, wp, bp, out, ccs):
    nc = tc.nc
    res = ctx.enter_context(tc.tile_pool(name="resident", bufs=1))

    # ---------- resident SBUF ----------
    qT = res.tile([128, BT], BF16)
    kT = res.tile([128, BT], BF16)
    vsb = res.tile([128, B * JTN, HPC, VW], BF16)     # [V | 1 | pad] per j-tile/head
    wq_sb = res.tile([128, KC, FPC], BF16)
    wk_sb = res.tile([128, KC, FPC], BF16)
    wv_sb = res.tile([128, KC, FPC], BF16)
    b_sb = res.tile([128, 3], F32)
    bv_sb = res.tile([128, 4, HPC, HD], F32)          # v-bias, free-axis broadcast
    wp_sb = res.tile([128, KC, C], BF16)
    bp_sb = res.tile([128, C], F32)
    yT0 = res.tile([64, BT], BF16)
    yT1 = res.tile([64, BT], BF16)
    yT = (yT0, yT1)

    # ---------- critical-path loads only; the rest queue after chunk 0 ----------
    nc.sync.dma_start(wq_sb[:], wq)
    nc.sync.dma_start(b_sb[:], bqkv.rearrange("b p -> p b"))

    ospool = ctx.enter_context(tc.tile_pool(name="osb", bufs=3))
    yfpool = ctx.enter_context(tc.tile_pool(name="yf", bufs=3))
    ph = ExitStack()
    xpool = ph.enter_context(tc.tile_pool(name="xt", bufs=3))
    gpool = ph.enter_context(tc.tile_pool(name="gps", bufs=2, space="PSUM"))
    spool = ph.enter_context(tc.tile_pool(name="sps", bufs=2, space="PSUM"))
    ypool = ph.enter_context(tc.tile_pool(name="yps", bufs=2, space="PSUM"))
    ptpool = ph.enter_context(tc.tile_pool(name="pt", bufs=3))
    npool = ph.enter_context(tc.tile_pool(name="norm", bufs=3))

    xT_t = xT.rearrange("(a p) t -> p a t", p=128)
    staging = [[] for _ in range(B)]

    def load_chunk(tci):
        t0 = tci * TCHUNK
        xt = xpool.tile([128, KC, TCHUNK], BF16, tag="xt", name=f"xt{tci}")
        for spl in range(4):
            nc.sync.dma_start(
                xt[:, 2 * spl : 2 * spl + 2, :],
                xT_t[:, 2 * spl : 2 * spl + 2, t0 : t0 + TCHUNK],
            )
        return xt

    def emit_qkv_chunk(tci, xt=None):
        """q^T/k^T (weights stationary) + natural-layout v (x^T stationary)
        for one 512-row chunk."""
        t0 = tci * TCHUNK
        if xt is None:
            xt = load_chunk(tci)
        for w_sb, bi, dst in ((wq_sb, 0, qT), (wk_sb, 1, kT)):
            ps = gpool.tile([128, TCHUNK], F32, tag="g", name=f"ps{tci}_{bi}")
            for a in range(KC):
                nc.tensor.matmul(
                    ps[:], lhsT=w_sb[:, a, :], rhs=xt[:, a, :],
                    start=(a == 0), stop=(a == KC - 1),
                )
            nc.vector.tensor_scalar_add(
                dst[:, t0 : t0 + TCHUNK], ps[:], b_sb[:, bi : bi + 1]
            )
        vp = gpool.tile([128, 4, HPC, HD], F32, tag="g", name=f"vp{tci}")
        for sub in range(4):
            for a in range(KC):
                nc.tensor.matmul(
                    vp[:, sub], lhsT=xt[:, a, sub * 128 : (sub + 1) * 128],
                    rhs=wv_sb[:, a, :], start=(a == 0), stop=(a == KC - 1),
                )
        g0 = tci * 4
        nc.vector.tensor_add(vsb[:, g0 : g0 + 4, :, 0:HD], vp[:], bv_sb[:])

    def emit_pv(b, j, pt, yps, njt):
        for h in range(HPC):
            nc.tensor.matmul(
                yps[h][:], lhsT=vsb[:, b * JTN + j, h, 0 : HD + 1],
                rhs=pt[:, h, :], start=(j == 0), stop=(j == njt - 1),
            )

    def emit_attn_block(b, qb):
        """Causal attention for one (batch, query-block): S^T for both heads
        into a 2-bank PSUM pair, one wide exp, PV one j behind S."""
        q0g = b * T + qb * QB
        njt = (qb + 1) * (QB // 128)
        yps = [
            ypool.tile([HD + 1, QB], F32, tag="yp", name=f"yp{b}_{qb}_{h}")
            for h in range(HPC)
        ]
        pts = []
        for j in range(njt):
            j0g = b * T + j * 128
            diag = j * 128 + 127 > qb * QB
            i0 = max(0, j * 128 - qb * QB)
            sp = spool.tile([128, HPC, QB], F32, tag="sp", name=f"sp{b}_{qb}_{j}")
            for h in range(HPC):
                hs = slice(h * HD, (h + 1) * HD)
                nc.tensor.matmul(
                    sp[:, h, i0:QB], lhsT=kT[hs, j0g : j0g + 128],
                    rhs=qT[hs, q0g + i0 : q0g + QB], start=True, stop=True,
                )
            pt = ptpool.tile([128, HPC, QB], BF16, tag="pt", name=f"pt{b}_{qb}_{j}")
            if i0:
                nc.vector.memset(pt[:, :, 0:i0], 0.0)
            nc.scalar.activation(
                pt[:, :, i0:QB], sp[:, :, i0:QB], AF.Exp, scale=float(SCALE)
            )
            if diag:
                for h in range(HPC):
                    nc.gpsimd.affine_select(
                        pt[:, h, i0 : i0 + 128], pt[:, h, i0 : i0 + 128],
                        pattern=[[1, 128]], base=0, channel_multiplier=-1,
                        compare_op=mybir.AluOpType.is_ge, fill=0.0,
                    )
            pts.append(pt)
            if j > 0:
                emit_pv(b, j - 1, pts[j - 1], yps, njt)
        emit_pv(b, njt - 1, pts[njt - 1], yps, njt)

        # softmax normalization (row HD of yp is the denominator), then stage
        # the two 256-row exchange slices of this block (plain DMA).
        cc_in = ccs[b][0]
        for h in range(HPC):
            ln = npool.tile([1, QB], F32, tag="ln")
            nc.vector.tensor_copy(ln[:], yps[h][HD : HD + 1, :])
            yraw = npool.tile([HD, QB], F32, tag="yraw")
            nc.vector.tensor_copy(yraw[:], yps[h][0:HD, :])
            rn = npool.tile([1, QB], F32, tag="rn")
            sc = npool.tile([1, QB], F32, tag="sc")
            nc.vector.reciprocal_approx_accurate(rn[:], ln[:], sc[:])
            rb = npool.tile([HD, QB], F32, tag="rb")
            nc.gpsimd.partition_broadcast(rb[:], rn[:], channels=HD)
            nc.vector.tensor_mul(yT[h][:, q0g : q0g + QB], yraw[:], rb[:])
            for half in range(2):
                ti = nc.sync.dma_start(
                    cc_in[2 * qb + half, h * HD : (h + 1) * HD, :],
                    yT[h][:, q0g + half * RPB : q0g + (half + 1) * RPB],
                )
                staging[b].append(ti)

    def emit_exchange(b):
        cc_in, cc_out = ccs[b]
        cci = nc.gpsimd.collective_compute(
            "AllToAll", mybir.AluOpType.bypass,
            ins=[cc_in[:]], outs=[cc_out[:]],
            replica_groups=[list(range(N_CORES))],
        )
        for ti in staging[b]:
            add_dep_helper(cci.ins, ti.ins, True, f"staging before exch{b}")

    pending_proj = []  # (yf_tile, out_row0) with gather DMA already issued

    def emit_proj_dma(b, half):
        """Prefetch the gathered 128-row y tile for proj tile (b, half)."""
        cc_out = ccs[b][1]
        yf = yfpool.tile([128, KC, 128], BF16, tag="yf", name=f"yf{b}_{half}")
        src = bass.AP(
            tensor=cc_out.tensor, offset=cc_out.offset + half * 128,
            ap=[[RPB, 128], [FPC * RPB, N_CORES], [1, 128]],
        )
        nc.sync.dma_start(yf[:], src)
        pending_proj.append((yf, b * RPB + half * 128))

    def emit_proj_mm(psum_pool, ptag):
        """Output projection for the oldest prefetched tile."""
        yf, row0 = pending_proj.pop(0)
        ps0 = psum_pool.tile([128, 512], F32, tag=ptag, name=f"op0_{row0}")
        ps1 = psum_pool.tile([128, 512], F32, tag=ptag, name=f"op1_{row0}")
        for a in range(KC):
            nc.tensor.matmul(ps0[:], lhsT=yf[:, a, :], rhs=wp_sb[:, a, 0:512],
                             start=(a == 0), stop=(a == KC - 1))
            nc.tensor.matmul(ps1[:], lhsT=yf[:, a, :], rhs=wp_sb[:, a, 512:C],
                             start=(a == 0), stop=(a == KC - 1))
        osb = ospool.tile([128, C], F32, tag="osb", name=f"osb{row0}")
        nc.vector.tensor_add(osb[:, 0:512], ps0[:], bp_sb[:, 0:512])
        nc.vector.tensor_add(osb[:, 512:C], ps1[:], bp_sb[:, 512:C])
        nc.sync.dma_start(out[row0 : row0 + 128, :], osb[:])

    # ---------- fused qkv + attention + exchange + projection pipeline ----------
    # batch 0's chunks interleave with its own attention (one chunk ahead);
    # later batches' chunks ride inside the previous batch's blocks.
    xt0 = load_chunk(0)
    nc.sync.dma_start(wk_sb[:], wk)
    nc.sync.dma_start(wv_sb[:], wv)
    bv_bcast = bass.AP(
        tensor=bqkv.tensor, offset=bqkv.offset + 2 * FPC,
        ap=[[0, 128], [0, 4], [HD, HPC], [1, HD]],
    )
    nc.sync.dma_start(bv_sb[:], bv_bcast)
    nc.vector.memset(vsb[:, :, :, HD : HD + 1], 1.0)
    emit_qkv_chunk(0, xt0)
    nc.sync.dma_start(wp_sb[:], wp)
    bp_bcast = bass.AP(tensor=bp.tensor, offset=bp.offset, ap=[[0, 128], [1, C]])
    nc.sync.dma_start(bp_sb[:], bp_bcast)

    # proj work scheduled into the attention blocks of batches 2 and 3:
    # (dma, mm) pairs per (b, qb) slot; dmas run one slot ahead of their mms.
    proj_slots = {
        (2, 0): ([(0, 0)], 0), (2, 1): ([(0, 1)], 1), (2, 2): ([(1, 0)], 1),
        (2, 3): ([(1, 1)], 1), (3, 0): ([], 1), (3, 1): ([(2, 0)], 0),
        (3, 2): ([(2, 1)], 1), (3, 3): ([], 1),
    }
    for b in range(B):
        for qb in range(NQB):
            if b == 0 and qb + 1 < NQB:
                emit_qkv_chunk(qb + 1)
            emit_attn_block(b, qb)
            if b + 1 < B:
                emit_qkv_chunk((b + 1) * NQB + qb)
            dmas, n_mm = proj_slots.get((b, qb), ([], 0))
            for pb, half in dmas:
                emit_proj_dma(pb, half)
            for _ in range(n_mm):
                emit_proj_mm(gpool, "g")
        emit_exchange(b)

    # tail: batch-3 exchange + its two projection tiles.  The attention-era
    # pools are closed first so each tile gets its own pair of PSUM banks.
    emit_proj_dma(3, 0)
    emit_proj_dma(3, 1)
    ph.close()
    opool = ctx.enter_context(tc.tile_pool(name="ops", bufs=2, space="PSUM"))
    emit_proj_mm(opool, "t0")
    emit_proj_mm(opool, "t1")
    assert not pending_proj


_COMPILED_NC = None


def _get_nc():
    global _COMPILED_NC
    if _COMPILED_NC is None:
        nc = bacc.Bacc("TRN2", target_bir_lowering=False, debug=False,
                       num_devices=N_CORES)
        build_program(nc)
        nc.compile()
    _COMPILED_NC = nc
    return _COMPILED_NC


def kernel(x, W_attn, b_attn, W_proj, b_proj):
    global LAST_RESULTS
    nc = _get_nc()

    bf = ml_dtypes.bfloat16
    xT_np = np.ascontiguousarray(
        np.asarray(x, np.float32).reshape(BT, C).T
    ).astype(bf)
    W_attn = np.asarray(W_attn, np.float32)
    b_attn = np.asarray(b_attn, np.float32)
    wp_np = np.ascontiguousarray(
        np.asarray(W_proj, np.float32).reshape(KC, 128, C).transpose(1, 0, 2)
    ).astype(bf)
    bp_np = np.asarray(b_proj, np.float32)

    def sb_layout(w, ncols):
        # [C, ncols] -> [128, KC, ncols]: partition-contiguous SBUF layout
        return np.ascontiguousarray(
            w.reshape(KC, 128, ncols).transpose(1, 0, 2)
        ).astype(bf)

    in_maps = []
    for c in range(N_CORES):
        s = slice(c * FPC, (c + 1) * FPC)
        in_maps.append({
            "xT": xT_np,
            "wq": sb_layout(W_attn[:, s], FPC),
            "wk": sb_layout(W_attn[:, C:2 * C][:, s], FPC),
            "wv": sb_layout(W_attn[:, 2 * C:][:, s], FPC),
            "bqkv": np.ascontiguousarray(
                np.stack([b_attn[s], b_attn[C:2 * C][s], b_attn[2 * C:][s]])
            ).astype(np.float32),
            "wp": wp_np,
            "bp": bp_np,
        })

    res = run_bass_kernel_spmd(nc, in_maps, core_ids=list(range(N_CORES)))
    LAST_RESULTS = res
    # Core r holds, per batch b, global rows [b*T + r*RPB, +RPB) at local
    # rows [b*RPB, +RPB).
    full = np.empty((BT, C), np.float32)
    for b in range(B):
        for r in range(N_CORES):
            full[b * T + r * RPB : b * T + (r + 1) * RPB] = (
                res.results[r]["out"][b * RPB : (b + 1) * RPB]
            )
    return full.reshape(B, T, C)
